# revision 3
# baseline (speedup 1.0000x reference)
"""Trainium2 Bass kernel for nn_LossKMeansWasserstein — redesign v2.

Host computes cluster membership, the filling loss, and a common eps0 =
max over all three cost matrices (deviation vs per-kind eps0 ~1e-6 rel).
The 24 per-cluster Sinkhorn problems are bin-packed across 8 cores
(balanced by an ACT-engine cost proxy).  Each core gets its OWN specialized
Bass module (no SPMD padding waste); the 8 single-core NEFFs are dispatched
asynchronously to the 8 jax devices and run concurrently.

Per half-iteration of one problem: V = h_j + x_i.y_j - 0.5|x_i|^2 is
recomputed by the PE from augmented vectors in f32r (1 cycle/row vs fp32's
4; abs err ~0.2 on |V|~1000 which propagates <1e-3 rel into the loss);
rowmax splits DVE/GPSIMD; exp+accum on ACT (Exp and Ln pinned to the one
activation table containing both, avoiding per-iteration table loads).
Final potentials DMA back; host takes means and assembles the loss.
"""
import os
import sys
from contextlib import ExitStack

import numpy as np

sys.path.insert(0, "/opt/trn_rl_repo")

import concourse.bass as bass  # noqa: E402
import concourse.tile as tile  # noqa: E402
from concourse import bacc, mybir  # noqa: E402

F32 = mybir.dt.float32
F32R = mybir.dt.float32r
AF = mybir.ActivationFunctionType
ALU = mybir.AluOpType
AXL = mybir.AxisListType

N, M, D, K = 3072, 3072, 64, 8
BLUR = 0.05
EPS = np.float32(BLUR ** 2)
SCAL2 = np.float32(0.8 ** 2)
NITER = int(os.environ.get("KM_NITER", "30"))
NSEQ = NITER + 1
NCORES = 8

_cache = {}


def _prefer_combined_act_table(arch="gen3"):
    """Make the act-table placement pass pick the table holding BOTH Exp and
    Ln so the per-iteration Exp/Ln alternation doesn't thrash table loads.
    Keeps dict order/length identical (act_func_set_id indexes the original
    list); only removes exp/ln from tables that don't hold both."""
    import concourse.hw_specs as hs
    tables = hs.get_activation_tables(arch)
    exp_fn = ln_fn = None
    for s in tables.values():
        for f in s:
            if f.name.lower() == "exp":
                exp_fn = f
            elif f.name.lower() == "ln":
                ln_fn = f
    if exp_fn is None or ln_fn is None:
        return
    both = [nm for nm, s in tables.items() if exp_fn in s and ln_fn in s]
    if not both:
        return
    for nm, s in tables.items():
        if nm not in both:
            s.discard(exp_fn)
            s.discard(ln_fn)


def _nb(v):
    return (v + 127) // 128


def _chunks(m):
    """Split m columns at 512 boundaries (matmul output must stay inside one
    PSUM bank; 512-col chunks also keep f32r at 1 cycle/row)."""
    return [(c0, min(c0 + 512, m)) for c0 in range(0, m, 512)]


def _pack(probs):
    """Greedy balance of problems over NCORES by ACT-cost proxy."""
    def cost(p):
        return (_nb(p["n"]) * p["m"] + _nb(p["m"]) * p["n"]
                + 461 * (_nb(p["n"]) + _nb(p["m"])))
    loads = [0.0] * NCORES
    cores = [[] for _ in range(NCORES)]
    for p in sorted(probs, key=cost, reverse=True):
        ci = int(np.argmin(loads))
        cores[ci].append(p)
        loads[ci] += cost(p)
    return cores


# --------------------------------------------------------------------------
# device program (one specialized module per core)
# --------------------------------------------------------------------------

def _build_core(sig, tsafe=-1):
    """sig: tuple of (n, m) per problem. tsafe: iterations t<=tsafe skip the
    rowmax (bias 0 is overflow-safe while eps_t is large).  Returns compiled
    single-core nc."""
    nc = bacc.Bacc("TRN2", target_bir_lowering=False, debug=False)
    _prefer_combined_act_table(nc.m.arch)

    probs = []
    bF = bG = cF = cG = 0          # block and column offsets
    for (n, m) in sig:
        me = m + (m & 1)           # fp32r needs even free widths; the pad
        ne = n + (n & 1)           # column's h stays at -1e6 (host-set)
        p = dict(n=n, m=m, me=me, ne=ne, nbn=_nb(n), nbm=_nb(m),
                 bF=bF, bG=bG, cF=cF, cG=cG)
        probs.append(p)
        bF += p["nbn"]
        bG += p["nbm"]
        cF += me                   # Vf columns = y side (even-padded)
        cG += ne                   # Vg columns = x side
    NBF, NBG, MT, NT = bF, bG, cF, cG
    MAXW = max(max(p["me"], p["ne"]) for p in probs)
    MAXNB = max(max(p["nbn"], p["nbm"]) for p in probs)

    d_uf = nc.dram_tensor("uf", [66, NBF * 128], F32, kind="ExternalInput").ap()
    d_vf = nc.dram_tensor("vf", [66, MT], F32, kind="ExternalInput").ap()
    d_ug = nc.dram_tensor("ug", [66, NBG * 128], F32, kind="ExternalInput").ap()
    d_vg = nc.dram_tensor("vg", [66, NT], F32, kind="ExternalInput").ap()
    d_hcf = nc.dram_tensor("hcf", [128, NSEQ * NBG], F32, kind="ExternalInput").ap()
    d_hcg = nc.dram_tensor("hcg", [128, NSEQ * NBF], F32, kind="ExternalInput").ap()
    d_ie = nc.dram_tensor("ie", [128, NSEQ], F32, kind="ExternalInput").ap()
    d_nie = nc.dram_tensor("nie", [128, NSEQ], F32, kind="ExternalInput").ap()
    d_ne = nc.dram_tensor("ne", [128, NSEQ], F32, kind="ExternalInput").ap()
    d_id = nc.dram_tensor("ident", [128, 128], F32, kind="ExternalInput").ap()
    d_out = nc.dram_tensor("fg2", [128, NBF + NBG], F32, kind="ExternalOutput").ap()

    with tile.TileContext(nc) as tc, ExitStack() as ctx:
        cp = ctx.enter_context(tc.tile_pool(name="cp", bufs=1))
        UF = cp.tile([66, NBF * 128], F32R)
        VF = cp.tile([66, MT], F32R)
        UG = cp.tile([66, NBG * 128], F32R)
        VG = cp.tile([66, NT], F32R)
        for t, d in ((UF, d_uf), (VF, d_vf), (UG, d_ug), (VG, d_vg)):
            nc.gpsimd.dma_start(t[:], d[:])
        hcf = cp.tile([128, NSEQ * NBG], F32)
        hcg = cp.tile([128, NSEQ * NBF], F32)
        ie = cp.tile([128, NSEQ], F32)
        nie = cp.tile([128, NSEQ], F32)
        ne = cp.tile([128, NSEQ], F32)
        ident = cp.tile([128, 128], F32)
        for t, d in ((hcf, d_hcf), (hcg, d_hcg), (ie, d_ie), (nie, d_nie),
                     (ne, d_ne), (ident, d_id)):
            nc.sync.dma_start(t[:], d[:])

        F = cp.tile([128, NBF], F32)
        G = cp.tile([128, NBG], F32)
        F2 = cp.tile([128, NBF], F32)
        G2 = cp.tile([128, NBG], F32)
        hf = cp.tile([128, NBG], F32)   # h for f-update (over y cols)
        hg = cp.tile([128, NBF], F32)
        # per-problem staging for transposed h rows (engine writes must
        # start at partition 0)
        stF = [cp.tile([p["nbm"], 128], F32R, name=f"stF{i}")
               for i, p in enumerate(probs)]
        stG = [cp.tile([p["nbn"], 128], F32R, name=f"stG{i}")
               for i, p in enumerate(probs)]
        m_f = cp.tile([128, NBF], F32)
        s_f = cp.tile([128, NBF], F32)
        b_f = cp.tile([128, NBF], F32)
        l_f = cp.tile([128, NBF], F32)
        m_g = cp.tile([128, NBG], F32)
        s_g = cp.tile([128, NBG], F32)
        b_g = cp.tile([128, NBG], F32)
        l_g = cp.tile([128, NBG], F32)

        # deeper matmul->exp pipelining when tiles fit one PSUM bank
        vbufs = 3 if MAXW * 4 <= 2048 else 2
        pv = ctx.enter_context(tc.tile_pool(name="pv", bufs=vbufs,
                                            space="PSUM"))
        ph = ctx.enter_context(tc.tile_pool(name="ph", bufs=2, space="PSUM"))
        pe = ctx.enter_context(tc.tile_pool(name="pe", bufs=1, space="PSUM"))

        nc.vector.memset(G[:], 0.0)

        # rotate tiny row0 DMAs across trigger queues so they parallelize
        dma_engines = [nc.sync, nc.gpsimd, nc.scalar]
        dq = [0]

        def half_update(dst, src, t, fside):
            """one potential update; fside: updating f (rows=x side).
            Per-problem chains are kept independent (separate h-add /
            transpose / Ln / assembly) so the tile scheduler can interleave
            them across engines."""
            if fside:
                U, V, hc, hv, stage = UF, VF, hcf, hf, stF
                mv, sv, bv, lv = m_f, s_f, b_f, l_f
            else:
                U, V, hc, hv, stage = UG, VG, hcg, hg, stG
                mv, sv, bv, lv = m_g, s_g, b_g, l_g
            NBc = NBG if fside else NBF
            use_max = t > tsafe
            for pi, p in enumerate(probs):
                nbc = p["nbm"] if fside else p["nbn"]
                boc = p["bG"] if fside else p["bF"]
                mcols = p["m"] if fside else p["n"]
                coff = p["cF"] if fside else p["cG"]
                stp = stage[pi]
                # h = src + (eps_t*logw - 0.5|pt|^2)   [Pool]
                nc.gpsimd.tensor_add(hv[:, boc:boc + nbc],
                                     src[:, boc:boc + nbc],
                                     hc[:, t * NBc + boc:t * NBc + boc + nbc])
                # transpose h blocks, stage as f32r, DMA into V row 0
                ptr = ph.tile([MAXNB, 128], F32, tag="ptr")
                nc.tensor.transpose(ptr[0:nbc, :], hv[:, boc:boc + nbc],
                                    ident[:])
                nc.vector.tensor_copy(stp[:], ptr[0:nbc, :])
                for b in range(nbc):
                    w = min(128, mcols - b * 128)
                    eng = dma_engines[dq[0] % len(dma_engines)]
                    dq[0] += 1
                    eng.dma_start(
                        V[0:1, coff + b * 128:coff + b * 128 + w],
                        stp[b:b + 1, 0:w])
            # per row block: matmul, rowmax [DVE], bias [DVE], exp+accum
            for p in probs:
                nbr = p["nbn"] if fside else p["nbm"]
                bor = p["bF"] if fside else p["bG"]
                mcols = p["me"] if fside else p["ne"]  # even-padded width
                coff = p["cF"] if fside else p["cG"]
                for b in range(nbr):
                    blk = bor + b
                    vps = pv.tile([128, MAXW], F32, tag="vps")
                    for (c0, c1) in _chunks(mcols):
                        nc.tensor.matmul(vps[:, c0:c1],
                                         U[:, blk * 128:(blk + 1) * 128],
                                         V[:, coff + c0:coff + c1])
                    expo = pe.tile([128, MAXW], F32, tag="expo")
                    if use_max:
                        nc.vector.tensor_reduce(mv[:, blk:blk + 1],
                                                vps[:, 0:mcols], AXL.X,
                                                ALU.max)
                        nc.vector.tensor_scalar_mul(bv[:, blk:blk + 1],
                                                    mv[:, blk:blk + 1],
                                                    nie[:, t:t + 1])
                        nc.scalar.activation(expo[:, 0:mcols],
                                             vps[:, 0:mcols], AF.Exp,
                                             bias=bv[:, blk:blk + 1],
                                             scale=ie[:, t:t + 1],
                                             accum_out=sv[:, blk:blk + 1])
                    else:
                        nc.scalar.activation(expo[:, 0:mcols],
                                             vps[:, 0:mcols], AF.Exp,
                                             scale=ie[:, t:t + 1],
                                             accum_out=sv[:, blk:blk + 1])
            # dst = -(m + eps*ln(s))   [ACT, then one Pool STT per problem]
            for p in probs:
                nbr = p["nbn"] if fside else p["nbm"]
                bor = p["bF"] if fside else p["bG"]
                sl = slice(bor, bor + nbr)
                nc.scalar.activation(lv[:, sl], sv[:, sl], AF.Ln)
                if use_max:
                    nc.vector.scalar_tensor_tensor(
                        dst[:, sl], lv[:, sl], ne[:, t:t + 1], mv[:, sl],
                        ALU.mult, ALU.subtract)
                else:
                    nc.vector.tensor_scalar_mul(dst[:, sl], lv[:, sl],
                                                ne[:, t:t + 1])

        for t in range(NITER):
            half_update(F, G, t, True)
            half_update(G, F, t, False)
        half_update(F2, G, NITER, True)
        half_update(G2, F, NITER, False)

        nc.sync.dma_start(d_out[:, 0:NBF], F2[:])
        nc.sync.dma_start(d_out[:, NBF:], G2[:])
    nc.compile()
    return nc


# --------------------------------------------------------------------------
# async heterogeneous multi-device runner (single-core path of
# bass2jax.run_bass_via_pjrt, minus the blocking np.asarray)
# --------------------------------------------------------------------------

def _make_runner(nc):
    import jax
    from concourse import bass2jax
    bass2jax.install_neuronx_cc_hook()
    assert not nc.dbg_callbacks
    partition_name = (nc.partition_id_tensor.name
                      if nc.partition_id_tensor else None)
    dbg_name = nc.dbg_addr.name if nc.dbg_addr is not None else None
    in_names, out_names, out_avals, zero_outs = [], [], [], []
    for alloc in nc.m.functions[0].allocations:
        if not isinstance(alloc, mybir.MemoryLocationSet):
            continue
        name = alloc.memorylocations[0].name
        if alloc.kind == "ExternalInput":
            if name != partition_name:
                in_names.append(name)
        elif alloc.kind == "ExternalOutput":
            out_names.append(name)
            shape = tuple(alloc.tensor_shape)
            dtype = mybir.dt.np(alloc.dtype)
            out_avals.append(jax.core.ShapedArray(shape, dtype))
            zero_outs.append(np.zeros(shape, dtype))
    n_params = len(in_names)
    all_names = in_names + out_names
    if partition_name is not None:
        all_names = all_names + [partition_name]
    donate = tuple(range(n_params, n_params + len(out_names)))

    def _body(*args):
        operands = list(args)
        if partition_name is not None:
            operands.append(bass2jax.partition_id_tensor())
        outs = bass2jax._bass_exec_p.bind(
            *operands,
            out_avals=tuple(out_avals),
            in_names=tuple(all_names),
            out_names=tuple(out_names),
            lowering_input_output_aliases=(),
            sim_require_finite=True,
            sim_require_nnan=True,
            nc=nc,
        )
        return tuple(outs)

    jitted = jax.jit(_body, donate_argnums=donate, keep_unused=True)
    return dict(jitted=jitted, in_names=in_names, out_names=out_names,
                zero_outs=zero_outs, dbg_name=dbg_name)


def _run_hetero(runners, in_maps):
    """Dispatch the 8 per-core programs asynchronously to the 8 devices.
    Non-donated input arrays are cached on-device across calls (the staged
    operands are deterministic functions of the kernel inputs)."""
    import time
    import zlib
    import jax
    h = 0
    for im in in_maps:
        for k in sorted(im):
            h = zlib.crc32(np.ascontiguousarray(im[k]).tobytes(), h)
    staged = None
    if _cache.get("staged_key") == h:
        staged = _cache.get("staged_args")
    if staged is None:
        _cache["staged_key"] = h
        staged = []
        for i, (r, im) in enumerate(zip(runners, in_maps)):
            dev = jax.devices()[i]
            im = dict(im)
            if r["dbg_name"] is not None:
                im[r["dbg_name"]] = np.zeros((1, 2), np.uint32)
            args = [jax.device_put(np.asarray(im[n]), dev)
                    for n in r["in_names"]]
            staged.append(args)
        _cache["staged_args"] = staged
    all_args = []
    for i, (r, args) in enumerate(zip(runners, staged)):
        dev = jax.devices()[i]
        zouts = [jax.device_put(z, dev) for z in r["zero_outs"]]
        all_args.append(args + zouts)
    t0 = time.time()
    futs = [r["jitted"](*args) for r, args in zip(runners, all_args)]
    for f in futs:
        for a in f:
            a.block_until_ready()
    _cache["exec_wall_ns"] = int((time.time() - t0) * 1e9)
    return [
        {n: np.asarray(a) for n, a in zip(r["out_names"], f)}
        for r, f in zip(runners, futs)
    ]


# --------------------------------------------------------------------------
# host orchestration
# --------------------------------------------------------------------------

def kernel(x, target, cluster_centers, filling_target, prediction_target):
    f32 = np.float32
    x = np.asarray(x, f32)
    target = np.asarray(target, f32)
    cluster_centers = np.asarray(cluster_centers, f32)
    filling_target = np.asarray(filling_target, f32)
    prediction_target = np.asarray(prediction_target)

    # ---- host: membership + filling loss ----
    nx = (x * x).sum(-1).astype(f32)
    ny = (target * target).sum(-1).astype(f32)
    ncc = (cluster_centers * cluster_centers).sum(-1).astype(f32)
    d_x = (nx[:, None] + ncc[None, :] - 2.0 * (x @ cluster_centers.T)).astype(f32)
    pred_x = d_x.argmin(1)
    s = -d_x.astype(np.float64)
    s -= s.max(1, keepdims=True)
    e = np.exp(s)
    filling_x = (e / e.sum(1, keepdims=True)).sum(0) / N
    loss_fil = np.mean((filling_x - filling_target.astype(np.float64)) ** 2)

    # ---- host: common eps0 = max over the three cost matrices ----
    gxy = x @ target.T
    mxy = float((0.5 * (nx[:, None] + ny[None, :] - 2.0 * gxy)).max())
    gxx = x @ x.T
    mxx = float((0.5 * (nx[:, None] + nx[None, :] - 2.0 * gxx)).max())
    gyy = target @ target.T
    myy = float((0.5 * (ny[:, None] + ny[None, :] - 2.0 * gyy)).max())
    del gxy, gxx, gyy
    eps0 = max(mxy, mxx, myy, float(EPS))

    # ---- problems & packing ----
    pts = {"x": x, "y": target}
    nrm = {"x": nx, "y": ny}
    probs = []
    for k in range(K):
        ix = np.where(pred_x == k)[0]
        iy = np.where(prediction_target == k)[0]
        if len(ix) == 0 or len(iy) == 0:
            continue
        probs.append(dict(n=len(ix), m=len(iy), ix=ix, iy=iy,
                          sx="x", sy="y", coeff=1.0))
        probs.append(dict(n=len(ix), m=len(ix), ix=ix, iy=ix,
                          sx="x", sy="x", coeff=-0.5))
        probs.append(dict(n=len(iy), m=len(iy), ix=iy, iy=iy,
                          sx="y", sy="y", coeff=-0.5))
    cores = _pack(probs)
    sigs = tuple(tuple((p["n"], p["m"]) for p in plist) for plist in cores)

    # iterations with eps_t >= 25 skip the rowmax (exp args stay in range;
    # see derivation in _build_core)
    tsafe = -1
    while (tsafe + 1 < NITER
           and eps0 * (float(SCAL2) ** (tsafe + 1)) >= 25.0):
        tsafe += 1

    # ---- compile (cached per layout) ----
    if ("mods", sigs, tsafe) not in _cache:
        mods = []
        for sig in sigs:
            key = ("mod", sig, tsafe)
            if key not in _cache:
                _cache[key] = _build_core(sig, tsafe)
            mods.append(_cache[key])
        _cache[("mods", sigs, tsafe)] = [_make_runner(m) for m in mods]
    runners = _cache[("mods", sigs, tsafe)]

    # ---- eps schedule (common) ----
    t_arr = np.arange(NITER, dtype=np.float64)
    seq = np.maximum(eps0 * (float(SCAL2) ** t_arr), float(EPS))
    seq = np.concatenate([seq, [float(EPS)]]).astype(f32)   # [NSEQ]
    ones128 = np.ones((128, 1), f32)
    ie_t = (ones128 * (1.0 / seq)[None, :]).astype(f32)
    nie_t = (-ie_t).astype(f32)
    ne_t = (ones128 * (-seq)[None, :]).astype(f32)
    ident = np.eye(128, dtype=f32)

    # ---- per-core inputs ----
    in_maps = []
    metas = []
    for plist in cores:
        NBF = sum(_nb(p["n"]) for p in plist)
        NBG = sum(_nb(p["m"]) for p in plist)
        MT = sum(p["m"] + (p["m"] & 1) for p in plist)
        NT = sum(p["n"] + (p["n"] & 1) for p in plist)
        uf = np.zeros((66, NBF * 128), f32)
        vf = np.zeros((66, MT), f32)
        ug = np.zeros((66, NBG * 128), f32)
        vg = np.zeros((66, NT), f32)
        vf[0, :] = -1e6   # pad columns' h never rewritten -> exp -> 0
        vg[0, :] = -1e6
        hcf = np.zeros((128, NSEQ * NBG), f32)
        hcg = np.zeros((128, NSEQ * NBF), f32)
        bF = bG = cF = cG = 0
        meta = []
        for p in plist:
            xp = pts[p["sx"]][p["ix"]]
            yp = pts[p["sy"]][p["iy"]]
            hx = 0.5 * nrm[p["sx"]][p["ix"]]
            hy = 0.5 * nrm[p["sy"]][p["iy"]]
            n, m = p["n"], p["m"]
            nbn, nbm = _nb(n), _nb(m)
            # f-update operands: U columns = x points, V columns = y points
            uf[0, bF * 128:bF * 128 + n] = 1.0
            uf[1:65, bF * 128:bF * 128 + n] = xp.T
            uf[65, bF * 128:bF * 128 + n] = -hx
            vf[1:65, cF:cF + m] = yp.T
            vf[65, cF:cF + m] = 1.0
            # g-update operands: U columns = y points, V columns = x points
            ug[0, bG * 128:bG * 128 + m] = 1.0
            ug[1:65, bG * 128:bG * 128 + m] = yp.T
            ug[65, bG * 128:bG * 128 + m] = -hy
            vg[1:65, cG:cG + n] = xp.T
            vg[65, cG:cG + n] = 1.0
            # h constants: hcf[j-block layout] = eps_t*log(1/m) - hy_j
            lb = np.float64(np.log(1.0 / m))
            la = np.float64(np.log(1.0 / n))
            for t in range(NSEQ):
                et = np.float64(seq[t])
                colf = np.full(nbm * 128, 0.0, np.float64)
                colf[:m] = et * lb - hy
                hcf[:, t * NBG + bG:t * NBG + bG + nbm] = \
                    colf.reshape(nbm, 128).T.astype(f32)
                colg = np.full(nbn * 128, 0.0, np.float64)
                colg[:n] = et * la - hx
                hcg[:, t * NBF + bF:t * NBF + bF + nbn] = \
                    colg.reshape(nbn, 128).T.astype(f32)
            meta.append(dict(n=n, m=m, bF=bF, bG=bG, coeff=p["coeff"]))
            bF += nbn
            bG += nbm
            cF += m + (m & 1)
            cG += n + (n & 1)
        in_maps.append({"uf": uf, "vf": vf, "ug": ug, "vg": vg,
                        "hcf": hcf, "hcg": hcg, "ie": ie_t, "nie": nie_t,
                        "ne": ne_t, "ident": ident})
        metas.append((meta, NBF, NBG))

    results = _run_hetero(runners, in_maps)
    _cache["last_results"] = results

    # ---- assemble loss ----
    loss_med = np.float64(0.0)
    for (meta, NBF, NBG), res in zip(metas, results):
        fg2 = res["fg2"].astype(np.float64)
        for p in meta:
            nbn, nbm = _nb(p["n"]), _nb(p["m"])
            f2 = fg2[:, p["bF"]:p["bF"] + nbn].T.reshape(-1)[:p["n"]]
            g2 = fg2[:, NBF + p["bG"]:NBF + p["bG"] + nbm].T.reshape(-1)[:p["m"]]
            loss_med += p["coeff"] * (f2.mean() + g2.mean())
    return np.asarray(f32(loss_fil + loss_med))
